# revision 57
# baseline (speedup 1.0000x reference)
"""Trainium2 Bass kernel: KV-memory retrieval (pool -> cosine kNN -> softmax gather).

Strategy (8 cores): shard the 65536-slot memory across cores (8192 keys/values
each) and the 256-image batch across cores (32 each) for pooling + output.
Keys are pre-transposed host-side to [C, M] so each core DMAs c-partitioned
kT tiles directly (no PE transposes for keys).

Per core, single SPMD launch:
  1. pool its x shard -> qT columns; two chunked AllGathers (first 16
     batches AG'd while the second half of x still streams in).
     Batch-tile permutation: tile A col r*16+j == global batch r*32+j,
     tile B col r*16+j == global batch r*32+16+j (undone at the mrow
     scatter before ReduceScatter).
  2. stream kT blocks [128c, 512m]: squares (ACT/DVE), norm via f32r
     ones-matmul over partitions, sqrt/recip, PE row-broadcast, DVE
     prescale -> normalized kTn (f32r); f32r matmul1 qT.T @ kTn ->
     sim [256, 8192]; per-block top-8 candidates (max8)
  3. local top-32 -> AllGather candidates -> global top-32 (sorted),
     threshold t, softmax stats gmax / Z (exp with per-partition
     scale/bias)
  4. dense w = exp(sim*rinv + bias) * (sim >= t)  (1/Z folded into bias),
     stored f32r
  5. matmul2 (f32r): values.T @ w -> partial matched.T; values are
     bitcast f32->f32r (no copy)
  6. transpose -> [256, 512], ReduceScatter(add) -> own batch shard
  7. broadcast over 784 spatial positions, DMA out [32, 512, 784]

f32r (single-pass fp32 on the PE) is safe here: measured sim noise ~1e-6
in cos units vs ~4e-4 gaps between rank 32/33. Selection is done on raw
r = q_sum . k_norm (scale-invariant per batch row); 1/||q|| enters only
through the exp scale. Mean /784 cancels everywhere.
"""

import math

import numpy as np

import concourse.bacc as bacc
import concourse.mybir as mybir
import concourse.tile as tile
from concourse.bass import ts
from concourse.bass_utils import run_bass_kernel_spmd
from concourse.masks import make_identity

F32 = mybir.dt.float32
F32R = mybir.dt.float32r
AF = mybir.ActivationFunctionType
ALU = mybir.AluOpType

N_CORES = 8
NEG = -3.0e38


def build(B=256, C=512, HW=784, M=65536, K=32, n_cores=N_CORES, mb=512):
    """Build + bacc-compile the SPMD program. Returns nc."""
    BS = B // n_cores          # batches per core
    HB = BS // 2               # half-batch chunk for split AllGather
    MS = M // n_cores          # memory slots per core
    CT = C // 128              # channel tiles (also contraction tiles)
    BT = B // 128 if B >= 128 else 1
    BTW = 128 if B >= 128 else B   # batch-tile width
    assert B % BTW == 0 and C % 128 == 0 and M % (n_cores * mb) == 0
    NMB = MS // mb             # key blocks per core
    R = math.ceil(K / 8)       # max8 rounds for exact top-K
    KPB = 8                    # top-8 per 512-block (validated sufficient)
    MT = MS // 128             # value tiles
    RG = [list(range(n_cores))]
    CC_AS = "Shared" if n_cores > 4 else "Local"

    nc = bacc.Bacc("TRN2", target_bir_lowering=False, debug=False,
                   num_devices=n_cores)

    xs = nc.dram_tensor("xs", [BS, C, HW], F32, kind="ExternalInput").ap()
    keysT = nc.dram_tensor("keysT", [C, MS], F32, kind="ExternalInput").ap()
    vals = nc.dram_tensor("vals", [MS, C], F32, kind="ExternalInput").ap()
    out = nc.dram_tensor("out", [BS, C, HW], F32, kind="ExternalOutput").ap()

    with tile.TileContext(nc) as tc:
        with (
            tc.tile_pool(name="consts", bufs=1) as consts,
            tc.tile_pool(name="persist", bufs=1) as persist,
            tc.tile_pool(name="dram", bufs=1, space="DRAM") as dram,
        ):
            identity = consts.tile([128, 128], F32)
            make_identity(nc, identity)
            identity_r = consts.tile([128, 128], F32R)
            nc.vector.tensor_copy(identity_r, identity)
            ones_col = consts.tile([128, 1], F32)
            nc.vector.memset(ones_col, 1.0)
            ones_col_r = consts.tile([128, 1], F32R)
            nc.vector.tensor_copy(ones_col_r, ones_col)
            ones_row = consts.tile([1, 128], F32)
            nc.vector.memset(ones_row, 1.0)
            ones_row_r = consts.tile([1, 128], F32R)
            nc.vector.tensor_copy(ones_row_r, ones_row)
            ones_hw = consts.tile([128, HW], F32)
            nc.vector.memset(ones_hw, 1.0)
            identity_h = consts.tile([128, 128], mybir.dt.bfloat16)
            nc.vector.tensor_copy(identity_h, identity)

            sim = [persist.tile([BTW, MS], F32, name=f"sim{i}")
                   for i in range(BT)]
            qTt = persist.tile([128, CT, B], F32, name="qTt")
            qT = [qTt[:, i] for i in range(CT)]
            qTr = persist.tile([128, CT, B], F32R, name="qTr")
            qTl = [persist.tile([128, BS], F32, name=f"qTl{i}")
                   for i in range(CT)]
            cand = [persist.tile([BTW, NMB * KPB], F32, name=f"cand{i}")
                    for i in range(BT)]
            rinv = [persist.tile([BTW, 1], F32, name=f"rinv{i}")
                    for i in range(BT)]
            bias2 = [persist.tile([BTW, 1], F32, name=f"bias2{i}")
                     for i in range(BT)]
            g32 = [persist.tile([BTW, R * 8], F32, name=f"g32{i}")
                   for i in range(BT)]
            mrow = [persist.tile([BTW, C], F32, name=f"mrow{i}")
                    for i in range(BT)]
            mTmy = [persist.tile([128, BS], F32, name=f"mTmy{i}")
                    for i in range(CT)]

            # AllGather staging (queries). A single AG after pooling: an
            # early chunked AG measurably steals x-read DMA bandwidth and
            # its second chunk lands late (rank skew), stalling the PE
            # queue; key_prep needs no queries, so it fills the AG window.
            qag_in = dram.tile([C, BS], F32, name="qag_in")
            qag_out = dram.tile([n_cores, C, BS], F32, addr_space=CC_AS,
                                name="qag_out")

            def ag_queries():
                for ct in range(CT):
                    nc.scalar.dma_start(out=qag_in[ts(ct, 128), :],
                                        in_=qTl[ct])
                nc.gpsimd.collective_compute(
                    "AllGather", ALU.bypass, replica_groups=RG,
                    ins=[qag_in.opt()], outs=[qag_out.opt()])

            def unpack_queries():
                # on the SP queue: SP would self-block on ktb slot pacing
                # here anyway, while ACT must keep running key_prep work
                # through the AllGather window
                for r in range(n_cores):
                    nc.sync.dma_start(
                        out=qTt[:, :, r * BS:(r + 1) * BS],
                        in_=qag_out[r].rearrange("(ct p) b -> p ct b",
                                                 p=128))
                nc.vector.tensor_copy(qTr, qTt)

            # ---------------- Phase P: pool x -> qT local ----------------
            with (
                tc.tile_pool(name="poolP", bufs=3) as pP,
            ):
                hw_a = 0
                for a in range(int(math.isqrt(HW)), 1, -1):
                    if HW % a == 0:
                        hw_a = a
                        break
                CTH = CT // 2
                for b in range(BS):
                    if b % 2 == 0:
                        xt2 = pP.tile([128, 2, CT, HW], F32, tag="xt2",
                                      bufs=3)
                        nc.sync.dma_start(
                            out=xt2,
                            in_=xs[b:b + 2].rearrange(
                                "b (ct p) hw -> p b ct hw", p=128))
                    xt = xt2[:, b % 2]
                    if hw_a > 1:
                        xp = pP.tile([128, CTH, HW // hw_a], F32, tag="xp")
                        nc.vector.tensor_reduce(
                            out=xp,
                            in_=xt[:, 0:CTH].rearrange(
                                "p ct (a b) -> p ct a b", a=HW // hw_a),
                            axis=mybir.AxisListType.X, op=ALU.add)
                        xq = pP.tile([128, CTH], F32, tag="xq")
                        nc.vector.tensor_reduce(
                            out=xq, in_=xp,
                            axis=mybir.AxisListType.X, op=ALU.add)
                    else:
                        xq = pP.tile([128, CTH], F32, tag="xq")
                        nc.vector.tensor_reduce(
                            out=xq, in_=xt[:, 0:CTH],
                            axis=mybir.AxisListType.X, op=ALU.add)
                    for ct in range(CTH):
                        nc.vector.tensor_copy(qTl[ct][:, b:b + 1],
                                              xq[:, ct:ct + 1])
                    for ct in range(CTH, CT):
                        xsc = pP.tile([128, HW], F32, tag="xsc")
                        nc.scalar.activation(
                            xsc, xt[:, ct], AF.Copy,
                            accum_out=qTl[ct][:, b:b + 1])
                ag_queries()

            # ---------------- Phase K: keys -> sim + block candidates -----
            # value prefetch pool spans K+W so DMA never starves after keys
            VB = 4                      # value tiles per DMA
            CW = 4                      # wexp chunk width (value tiles)
            pV_cm = tc.tile_pool(name="poolV", bufs=2)
            pV = pV_cm.__enter__()
            vgroups = {}

            def issue_vals(g):
                vtb = pV.tile([128, VB, C], F32R, tag="vtb", bufs=2)
                nc.sync.dma_start(
                    out=vtb,
                    in_=vals[g * VB * 128:(g + 1) * VB * 128].rearrange(
                        "(v p) c -> p v c", p=128).bitcast(F32R))
                vgroups[g] = vtb

            with (
                tc.tile_pool(name="poolK", bufs=2) as pK,
                tc.tile_pool(name="psumK", bufs=1, space="PSUM") as psK,
            ):
                # Two passes over the keys. Pass 1 streams all 16 blocks
                # once to compute per-key 1/||k|| broadcast tiles (psbS_all)
                # — this has no dependency on the queries, so it fills the
                # entire AllGather window with DMA + PE + ACT/DVE work.
                # Pass 2 re-streams the keys (DMA has spare capacity here)
                # for a short, dependency-free sim pipeline.
                psbS_all = pK.tile([128, NMB, mb], F32, name="psbS_all",
                                   bufs=1)
                # per-block key norms land as rows of normsAll; ONE batched
                # DVE reciprocal then serves all blocks (DVE reciprocal cost
                # scales with free size only: [1,512] costs the same 3.3us
                # as [16,512], so per-block recips would burn 53us of DVE)
                rinvAll = pK.tile([NMB, mb], F32, name="rinvAll", bufs=1)
                # eyerow[0, mbi*NMB+k] == (k==mbi): one-hot rows, all on
                # partition 0, for packing norm rows into PSUM partitions
                eyerow = pK.tile([1, NMB * NMB], F32, name="eyerow", bufs=1)
                nc.vector.memset(eyerow, 0.0)
                for mbi in range(NMB):
                    nc.vector.memset(
                        eyerow[0:1, mbi * NMB + mbi:mbi * NMB + mbi + 1],
                        1.0)
                psnPack = psK.tile([NMB, mb], F32, name="psnPack", bufs=1)
                # one-hot selectors: sel_all[k, mbi*128+p] == (k == mbi), so
                # matmul(lhsT=sel_all[:, mbi-slice], rhs=rinvAll) replicates
                # row mbi across all 128 partitions exactly (fp32, 1.0*x)
                sel_all = pK.tile([NMB, NMB * 128], F32, name="sel_all",
                                  bufs=1)
                for mbi in range(NMB):
                    nc.vector.tensor_scalar_mul(
                        sel_all[:, ts(mbi, 128)], ones_hw[0:NMB, 0:128],
                        identity[0:NMB, mbi:mbi + 1])

                def key_norms(mbi):
                    ktb = pK.tile([128, CT, mb], F32, tag="ktb1", bufs=2)
                    nc.sync.dma_start(
                        out=ktb,
                        in_=keysT[:, mbi * mb:(mbi + 1) * mb].rearrange(
                            "(ct p) m -> p ct m", p=128))
                    # squares -> ksq (f32r-rounded, so the f32r norm matmul
                    # sums exactly-representable terms: ~8e-6 norm error)
                    ksq = pK.tile([128, CT, mb], F32R, tag="ksq", bufs=2)
                    for ct in range(CT):
                        if ct == 0:
                            nc.vector.tensor_mul(ksq[:, ct], ktb[:, ct],
                                                 ktb[:, ct])
                        else:
                            nc.scalar.square(ksq[:, ct], ktb[:, ct])
                    # norms^2: contract partitions via ones f32r matmul
                    psn = psK.tile([1, mb], F32, tag="psn", bufs=2)
                    for ct in range(CT):
                        nc.tensor.matmul(psn, lhsT=ones_col_r,
                                         rhs=ksq[:, ct],
                                         start=(ct == 0), stop=(ct == CT - 1),
                                         skip_group_check=True)
                    nrow = pK.tile([1, mb], F32, tag="nrow", bufs=2)
                    nc.scalar.sqrt(nrow, psn)
                    # stack row mbi of the norm matrix via an exact fp32
                    # outer product with a one-hot column
                    nc.tensor.matmul(
                        psnPack, lhsT=eyerow[0:1, ts(mbi, NMB)], rhs=nrow,
                        start=(mbi == 0), stop=(mbi == NMB - 1),
                        skip_group_check=True)

                def bcast_rinv(mbi):
                    # broadcast rinv row to 128 partitions on the PE in FULL
                    # fp32 (1.0 * x is exact; f32r here would round the
                    # per-key scale and flip rank-32/33 boundaries)
                    psb = psK.tile([128, mb], F32, tag="psb", bufs=2)
                    nc.tensor.matmul(psb,
                                     lhsT=sel_all[:, ts(mbi, 128)],
                                     rhs=rinvAll,
                                     start=True, stop=True,
                                     skip_group_check=True)
                    nc.scalar.copy(psbS_all[:, mbi], psb)

                def sim_block(mbi, ktb):
                    for bt in range(BT):
                        psim = psK.tile([BTW, mb], F32, tag="psim", bufs=3)
                        for dt in range(CT):
                            nc.tensor.matmul(
                                psim, lhsT=qTr[:, dt, ts(bt, BTW)],
                                rhs=ktb[:, dt],
                                start=(dt == 0), stop=(dt == CT - 1),
                                skip_group_check=True)
                        # evict + normalize by 1/||k|| in one DVE op
                        sblk = sim[bt][:, ts(mbi, mb)]
                        nc.vector.tensor_mul(sblk, psim, psbS_all[:, mbi])
                        nc.vector.max(
                            cand[bt][:, mbi * KPB:mbi * KPB + KPB], sblk)

                for mbi in range(NMB):
                    key_norms(mbi)
                nc.vector.reciprocal(rinvAll, psnPack)
                for mbi in range(NMB):
                    bcast_rinv(mbi)
                unpack_queries()
                ktb2s = {}

                def key_load2(mbi):
                    ktb2 = pK.tile([128, CT, mb], F32R, tag="ktb2", bufs=3)
                    nc.sync.dma_start(
                        out=ktb2,
                        in_=keysT[:, mbi * mb:(mbi + 1) * mb].rearrange(
                            "(ct p) m -> p ct m", p=128).bitcast(F32R))
                    ktb2s[mbi] = ktb2

                PF = 2
                for mbi in range(PF):
                    key_load2(mbi)
                for mbi in range(NMB):
                    if mbi + PF < NMB:
                        key_load2(mbi + PF)
                    if mbi >= NMB - 4:
                        issue_vals(mbi - (NMB - 4))
                    sim_block(mbi, ktb2s.pop(mbi))

            # ---------------- Phase Q: query norms ----------------
            with (
                tc.tile_pool(name="poolQ", bufs=2) as pQ,
                tc.tile_pool(name="psumQ", bufs=1, space="PSUM") as psQ,
            ):
                psum_ssq = psQ.tile([1, B], F32, tag="ssq")
                for ct in range(CT):
                    qsq = pQ.tile([128, B], F32R, tag="qsq")
                    nc.scalar.square(qsq, qT[ct])
                    nc.tensor.matmul(psum_ssq, lhsT=ones_col_r, rhs=qsq,
                                     start=(ct == 0), stop=(ct == CT - 1))
                qn_row = pQ.tile([1, B], F32, tag="qn_row", bufs=1)
                nc.scalar.sqrt(qn_row, psum_ssq)
                ri_row = pQ.tile([1, B], F32, tag="ri_row", bufs=1)
                nc.vector.reciprocal(ri_row, qn_row)
                for bt in range(BT):
                    psum_rt = psQ.tile([BTW, 1], F32, tag="rt")
                    nc.tensor.matmul(
                        psum_rt, lhsT=ri_row[0:1, ts(bt, BTW)],
                        rhs=ones_col[0:1, 0:1], start=True, stop=True)
                    nc.vector.tensor_copy(rinv[bt], psum_rt)

            # ---------------- Phase G + W: top-K stats, dense matmul2 -----
            cd_in = dram.tile([B, K], F32)
            cd_out = dram.tile([n_cores, B, K], F32, addr_space=CC_AS)
            with (
                tc.tile_pool(name="poolW", bufs=2) as pW,
                tc.tile_pool(name="psumW", bufs=1, space="PSUM") as psW,
                tc.tile_pool(name="poolG", bufs=1) as pG,
            ):
                # G1: local top-K per bt -> AllGather (bt chains interleaved
                # so the DVE pipeline stays full)
                locs = [pG.tile([BTW, R * 8], F32, tag=f"loc{bt}",
                                name=f"loc{bt}") for bt in range(BT)]
                scr2s = [pG.tile([BTW, NMB * KPB], F32, tag=f"scr2{bt}",
                                 name=f"scr2{bt}") for bt in range(BT)]
                curs = [cand[bt] for bt in range(BT)]
                for r in range(R):
                    for bt in range(BT):
                        nc.vector.max(locs[bt][:, r * 8:(r + 1) * 8],
                                      curs[bt])
                    if r < R - 1:
                        for bt in range(BT):
                            nc.vector.match_replace(
                                scr2s[bt],
                                in_to_replace=locs[bt][:, r * 8:(r + 1) * 8],
                                in_values=curs[bt], imm_value=NEG)
                            curs[bt] = scr2s[bt]
                for bt in range(BT):
                    nc.sync.dma_start(out=cd_in[ts(bt, BTW), :],
                                      in_=locs[bt][:, 0:K])
                nc.gpsimd.collective_compute(
                    "AllGather", ALU.bypass, replica_groups=RG,
                    ins=[cd_in.opt()], outs=[cd_out.opt()])

                issue_vals(4)
                issue_vals(5)

                # G2: global top-K + softmax stats. bt chains interleaved;
                # Exp/Ln activations grouped to avoid ACT table thrash.
                gcs, scr3s = [], []
                for bt in range(BT):
                    gc = pG.tile([BTW, n_cores * K], F32, tag=f"gc{bt}",
                                 name=f"gc{bt}")
                    nc.scalar.dma_start(
                        out=gc,
                        in_=cd_out[:, ts(bt, BTW), :].rearrange(
                            "r b k -> b r k"))
                    gcs.append(gc)
                    scr3s.append(pG.tile([BTW, n_cores * K], F32,
                                         tag=f"scr3{bt}", name=f"scr3{bt}"))
                curs = gcs
                for r in range(R):
                    for bt in range(BT):
                        nc.vector.max(g32[bt][:, r * 8:(r + 1) * 8],
                                      curs[bt])
                    if r < R - 1:
                        for bt in range(BT):
                            nc.vector.match_replace(
                                scr3s[bt],
                                in_to_replace=g32[bt][:, r * 8:(r + 1) * 8],
                                in_values=curs[bt], imm_value=NEG)
                        curs = scr3s
                # stats: nb = -gmax*rinv ; Z = sum exp((g - gmax)*rinv)
                nbs, zzs = [], []
                for bt in range(BT):
                    nb = pG.tile([BTW, 1], F32, tag=f"nb{bt}",
                                 name=f"nb{bt}")
                    nc.vector.tensor_mul(nb, g32[bt][:, 0:1], rinv[bt])
                    nc.vector.tensor_scalar_mul(nb, nb, -1.0)
                    nbs.append(nb)
                for bt in range(BT):
                    ex = pG.tile([BTW, K], F32, tag="ex")
                    zz = pG.tile([BTW, 1], F32, tag=f"zz{bt}",
                                 name=f"zz{bt}")
                    nc.scalar.activation(ex, g32[bt][:, 0:K], AF.Exp,
                                         bias=nbs[bt], scale=rinv[bt],
                                         accum_out=zz)
                    zzs.append(zz)
                lnzs = []
                for bt in range(BT):
                    lnz = pG.tile([BTW, 1], F32, tag=f"lnz{bt}",
                                  name=f"lnz{bt}")
                    nc.scalar.activation(lnz, zzs[bt], AF.Ln)
                    lnzs.append(lnz)
                for bt in range(BT):
                    nc.vector.tensor_sub(bias2[bt], nbs[bt], lnzs[bt])

                # W: dense weights + matmul2
                pmB = [psW.tile([BTW, C], F32, tag=f"pmB{bt}",
                                name=f"pmB{bt}") for bt in range(BT)]
                for mt in range(MT):
                    g = mt // VB
                    if mt % VB == 0 and g + 3 not in vgroups and \
                            (g + 3) * VB * 128 < MS:
                        issue_vals(g + 3)
                    if mt % CW == 0:
                        # weights quantized to bf16 after the f32 threshold
                        # compare: 2^-9 relative on softmax weights is far
                        # below the tolerance, and bf16 transposes run the
                        # PE at 1 cyc/row with fast (FWL) weight loads.
                        weF = [pW.tile([BTW, CW * 128], F32R, tag=f"weF{bt}",
                                       bufs=2, name=f"weF{bt}")
                               for bt in range(BT)]
                        we = [pW.tile([BTW, CW * 128], mybir.dt.bfloat16,
                                      tag=f"we{bt}", bufs=2,
                                      name=f"we{bt}")
                              for bt in range(BT)]
                        for bt in range(BT):
                            schunk = sim[bt][:, mt * 128:(mt + CW) * 128]
                            nc.scalar.activation(weF[bt], schunk, AF.Exp,
                                                 bias=bias2[bt],
                                                 scale=rinv[bt])
                            nc.vector.scalar_tensor_tensor(
                                out=we[bt], in0=schunk,
                                scalar=g32[bt][:, K - 1:K], in1=weF[bt],
                                op0=ALU.is_ge, op1=ALU.mult)
                    vt = vgroups[g][:, mt % VB]
                    off = (mt % CW) * 128
                    pwt = psW.tile([128, B], mybir.dt.bfloat16, tag="pwt",
                                   bufs=4)
                    for bt in range(BT):
                        nc.tensor.matmul(
                            pwt[:, ts(bt, BTW)],
                            lhsT=we[bt][:, off:off + 128],
                            rhs=identity_h[0:BTW, 0:BTW], is_transpose=True,
                            start=True, stop=True, skip_group_check=True)
                    wT = pW.tile([128, B], F32R, tag="wT", bufs=3)
                    if mt % 2 == 0:
                        nc.vector.tensor_copy(wT, pwt)
                    else:
                        nc.scalar.copy(wT, pwt)
                    for bt in range(BT):
                        nc.tensor.matmul(
                            pmB[bt], lhsT=wT[:, ts(bt, BTW)], rhs=vt,
                            start=(mt == 0), stop=(mt == MT - 1),
                            skip_group_check=True)
                for bt in range(BT):
                    nc.any.tensor_copy(mrow[bt], pmB[bt])
            pV_cm.__exit__(None, None, None)

            # ---------------- Phase O: reduce-scatter + broadcast out -----
            # two C-halves: RS of half B overlaps the broadcast + write of
            # half A, shortening the serial tail
            CH = C // 2
            CTH2 = CT // 2
            mb_dram = [dram.tile([B, CH], F32, name=f"mb_dram{h}")
                       for h in range(2)]
            rs_out = [dram.tile([BS, CH], F32, name=f"rs_out{h}")
                      for h in range(2)]
            with (
                tc.tile_pool(name="poolO", bufs=2) as pO,
                tc.tile_pool(name="psumO", bufs=1, space="PSUM") as psO,
            ):
                for h in range(2):
                    for bt in range(BT):
                        nc.sync.dma_start(
                            out=mb_dram[h][ts(bt, BTW), :],
                            in_=mrow[bt][:, h * CH:(h + 1) * CH])
                    nc.gpsimd.collective_compute(
                        "ReduceScatter", ALU.add, replica_groups=RG,
                        ins=[mb_dram[h].opt()], outs=[rs_out[h].opt()])
                mmy = [pO.tile([BS, CH], F32, tag=f"mmy{h}",
                               name=f"mmy{h}", bufs=1) for h in range(2)]
                for h in range(2):
                    nc.scalar.dma_start(out=mmy[h], in_=rs_out[h])
                    for dt2 in range(CTH2):
                        dt = h * CTH2 + dt2
                        pmt = psO.tile([128, BS], F32, tag="pmt", bufs=2)
                        nc.tensor.matmul(
                            pmt, lhsT=mmy[h][:, ts(dt2, 128)],
                            rhs=identity[0:BS, 0:BS], is_transpose=True,
                            start=True, stop=True, skip_group_check=True)
                        nc.any.tensor_copy(mTmy[dt], pmt)
                    for b2 in range(BS // 2):
                        ot = pO.tile([128, 2, CTH2, HW], F32, tag="ot",
                                     bufs=3)
                        for bb in range(2):
                            b = 2 * b2 + bb
                            for dt2 in range(CTH2):
                                dt = h * CTH2 + dt2
                                col = mTmy[dt][:, b:b + 1]
                                if (dt + bb) % 2 == 0:
                                    nc.vector.tensor_scalar_mul(
                                        ot[:, bb, dt2], ones_hw, col)
                                else:
                                    nc.scalar.mul(ot[:, bb, dt2], ones_hw,
                                                  col)
                        for bb in range(2):
                            nc.sync.dma_start(
                                out=out[2 * b2 + bb,
                                        h * CH:(h + 1) * CH].rearrange(
                                    "(ct p) hw -> p ct hw", p=128),
                                in_=ot[:, bb])

    nc.compile()
    return nc


_CACHE = {}
TRACE = False
TRACE_DIR = None
LAST_RESULT = None


def _get(shape_key):
    if shape_key not in _CACHE:
        _CACHE[shape_key] = build(*shape_key)
    return _CACHE[shape_key]


def kernel(x, keys, values, topk, **_ignored):
    K = int(np.asarray(topk))
    B, C, H, W = x.shape
    M, D = keys.shape
    HW = H * W
    nc = _get((B, C, HW, M, K, N_CORES))
    BS, MS = B // N_CORES, M // N_CORES
    x3 = np.ascontiguousarray(x.reshape(B, C, HW)).astype(np.float32,
                                                          copy=False)
    keysT = np.ascontiguousarray(keys.T).astype(np.float32, copy=False)
    values = np.ascontiguousarray(values).astype(np.float32, copy=False)
    in_maps = [{
        "xs": x3[c * BS:(c + 1) * BS],
        "keysT": np.ascontiguousarray(keysT[:, c * MS:(c + 1) * MS]),
        "vals": values[c * MS:(c + 1) * MS],
    } for c in range(N_CORES)]
    global LAST_RESULT
    res = run_bass_kernel_spmd(nc, in_maps, core_ids=list(range(N_CORES)),
                               trace=TRACE, tmpdir=TRACE_DIR)
    LAST_RESULT = res
    outs = [res.results[c]["out"] for c in range(N_CORES)]
    return np.concatenate(outs, axis=0).reshape(B, C, H, W)


# revision 58
# speedup vs baseline: 1.1384x; 1.1384x over previous
"""Trainium2 Bass kernel: KV-memory retrieval (pool -> cosine kNN -> softmax gather).

Strategy (8 cores): shard the 65536-slot memory across cores (8192 keys/values
each) and the 256-image batch across cores (32 each) for pooling + output.
Keys are pre-transposed host-side to [C, M] so each core DMAs c-partitioned
kT tiles directly (no PE transposes for keys).

Per core, single SPMD launch:
  1. pool its x shard -> qT columns; two chunked AllGathers (first 16
     batches AG'd while the second half of x still streams in).
     Batch-tile permutation: tile A col r*16+j == global batch r*32+j,
     tile B col r*16+j == global batch r*32+16+j (undone at the mrow
     scatter before ReduceScatter).
  2. stream kT blocks [128c, 512m]: squares (ACT/DVE), norm via f32r
     ones-matmul over partitions, sqrt/recip, PE row-broadcast, DVE
     prescale -> normalized kTn (f32r); f32r matmul1 qT.T @ kTn ->
     sim [256, 8192]; per-block top-8 candidates (max8)
  3. local top-32 -> AllGather candidates -> global top-32 (sorted),
     threshold t, softmax stats gmax / Z (exp with per-partition
     scale/bias)
  4. dense w = exp(sim*rinv + bias) * (sim >= t)  (1/Z folded into bias),
     stored f32r
  5. matmul2 (f32r): values.T @ w -> partial matched.T; values are
     bitcast f32->f32r (no copy)
  6. transpose -> [256, 512], ReduceScatter(add) -> own batch shard
  7. broadcast over 784 spatial positions, DMA out [32, 512, 784]

f32r (single-pass fp32 on the PE) is safe here: measured sim noise ~1e-6
in cos units vs ~4e-4 gaps between rank 32/33. Selection is done on raw
r = q_sum . k_norm (scale-invariant per batch row); 1/||q|| enters only
through the exp scale. Mean /784 cancels everywhere.
"""

import math

import numpy as np

import concourse.bacc as bacc
import concourse.mybir as mybir
import concourse.tile as tile
from concourse.bass import ts
from concourse.bass_utils import run_bass_kernel_spmd
from concourse.masks import make_identity

F32 = mybir.dt.float32
F32R = mybir.dt.float32r
AF = mybir.ActivationFunctionType
ALU = mybir.AluOpType

N_CORES = 8
NEG = -3.0e38


def build(B=256, C=512, HW=784, M=65536, K=32, n_cores=N_CORES, mb=512):
    """Build + bacc-compile the SPMD program. Returns nc."""
    BS = B // n_cores          # batches per core
    HB = BS // 2               # half-batch chunk for split AllGather
    MS = M // n_cores          # memory slots per core
    CT = C // 128              # channel tiles (also contraction tiles)
    BT = B // 128 if B >= 128 else 1
    BTW = 128 if B >= 128 else B   # batch-tile width
    assert B % BTW == 0 and C % 128 == 0 and M % (n_cores * mb) == 0
    NMB = MS // mb             # key blocks per core
    R = math.ceil(K / 8)       # max8 rounds for exact top-K
    KPB = 8                    # top-8 per 512-block (validated sufficient)
    MT = MS // 128             # value tiles
    RG = [list(range(n_cores))]
    CC_AS = "Shared" if n_cores > 4 else "Local"

    nc = bacc.Bacc("TRN2", target_bir_lowering=False, debug=False,
                   num_devices=n_cores)

    xs = nc.dram_tensor("xs", [BS, C, HW], F32, kind="ExternalInput").ap()
    keysT = nc.dram_tensor("keysT", [C, MS], F32, kind="ExternalInput").ap()
    vals = nc.dram_tensor("vals", [MS, C], F32, kind="ExternalInput").ap()
    out = nc.dram_tensor("out", [BS, C, HW], F32, kind="ExternalOutput").ap()

    with tile.TileContext(nc) as tc:
        with (
            tc.tile_pool(name="consts", bufs=1) as consts,
            tc.tile_pool(name="persist", bufs=1) as persist,
            tc.tile_pool(name="dram", bufs=1, space="DRAM") as dram,
        ):
            identity = consts.tile([128, 128], F32)
            make_identity(nc, identity)
            identity_r = consts.tile([128, 128], F32R)
            nc.vector.tensor_copy(identity_r, identity)
            ones_col = consts.tile([128, 1], F32)
            nc.vector.memset(ones_col, 1.0)
            ones_col_r = consts.tile([128, 1], F32R)
            nc.vector.tensor_copy(ones_col_r, ones_col)
            ones_row = consts.tile([1, 128], F32)
            nc.vector.memset(ones_row, 1.0)
            ones_row_r = consts.tile([1, 128], F32R)
            nc.vector.tensor_copy(ones_row_r, ones_row)
            ones_hw = consts.tile([128, HW], F32)
            nc.vector.memset(ones_hw, 1.0)
            identity_h = consts.tile([128, 128], mybir.dt.bfloat16)
            nc.vector.tensor_copy(identity_h, identity)

            sim = [persist.tile([BTW, MS], F32, name=f"sim{i}")
                   for i in range(BT)]
            qTt = persist.tile([128, CT, B], F32, name="qTt")
            qT = [qTt[:, i] for i in range(CT)]
            qTr = persist.tile([128, CT, B], F32R, name="qTr")
            qTl = [persist.tile([128, BS], F32, name=f"qTl{i}")
                   for i in range(CT)]
            cand = [persist.tile([BTW, NMB * KPB], F32, name=f"cand{i}")
                    for i in range(BT)]
            rinv = [persist.tile([BTW, 1], F32, name=f"rinv{i}")
                    for i in range(BT)]
            bias2 = [persist.tile([BTW, 1], F32, name=f"bias2{i}")
                     for i in range(BT)]
            g32 = [persist.tile([BTW, R * 8], F32, name=f"g32{i}")
                   for i in range(BT)]
            mrow = [persist.tile([BTW, C], F32, name=f"mrow{i}")
                    for i in range(BT)]
            mTmy = [persist.tile([128, BS], F32, name=f"mTmy{i}")
                    for i in range(CT)]

            # AllGather staging (queries). A single AG after pooling: an
            # early chunked AG measurably steals x-read DMA bandwidth and
            # its second chunk lands late (rank skew), stalling the PE
            # queue; key_prep needs no queries, so it fills the AG window.
            qag_in = dram.tile([C, BS], F32, name="qag_in")
            qag_out = dram.tile([n_cores, C, BS], F32, addr_space=CC_AS,
                                name="qag_out")

            def ag_queries():
                for ct in range(CT):
                    nc.scalar.dma_start(out=qag_in[ts(ct, 128), :],
                                        in_=qTl[ct])
                nc.gpsimd.collective_compute(
                    "AllGather", ALU.bypass, replica_groups=RG,
                    ins=[qag_in.opt()], outs=[qag_out.opt()])

            def unpack_queries():
                # on the SP queue: SP would self-block on ktb slot pacing
                # here anyway, while ACT must keep running key_prep work
                # through the AllGather window
                for r in range(n_cores):
                    nc.sync.dma_start(
                        out=qTt[:, :, r * BS:(r + 1) * BS],
                        in_=qag_out[r].rearrange("(ct p) b -> p ct b",
                                                 p=128))
                nc.vector.tensor_copy(qTr, qTt)

            # ---------------- Phase P: pool x -> qT local ----------------
            with (
                tc.tile_pool(name="poolP", bufs=3) as pP,
            ):
                hw_a = 0
                for a in range(int(math.isqrt(HW)), 1, -1):
                    if HW % a == 0:
                        hw_a = a
                        break
                CTH = CT // 2
                for b in range(BS):
                    if b % 2 == 0:
                        xt2 = pP.tile([128, 2, CT, HW], F32, tag="xt2",
                                      bufs=3)
                        nc.sync.dma_start(
                            out=xt2,
                            in_=xs[b:b + 2].rearrange(
                                "b (ct p) hw -> p b ct hw", p=128))
                    xt = xt2[:, b % 2]
                    if hw_a > 1:
                        xp = pP.tile([128, CTH, HW // hw_a], F32, tag="xp")
                        nc.vector.tensor_reduce(
                            out=xp,
                            in_=xt[:, 0:CTH].rearrange(
                                "p ct (a b) -> p ct a b", a=HW // hw_a),
                            axis=mybir.AxisListType.X, op=ALU.add)
                        xq = pP.tile([128, CTH], F32, tag="xq")
                        nc.vector.tensor_reduce(
                            out=xq, in_=xp,
                            axis=mybir.AxisListType.X, op=ALU.add)
                    else:
                        xq = pP.tile([128, CTH], F32, tag="xq")
                        nc.vector.tensor_reduce(
                            out=xq, in_=xt[:, 0:CTH],
                            axis=mybir.AxisListType.X, op=ALU.add)
                    for ct in range(CTH):
                        nc.vector.tensor_copy(qTl[ct][:, b:b + 1],
                                              xq[:, ct:ct + 1])
                    for ct in range(CTH, CT):
                        xsc = pP.tile([128, HW], F32, tag="xsc")
                        nc.scalar.activation(
                            xsc, xt[:, ct], AF.Copy,
                            accum_out=qTl[ct][:, b:b + 1])
                ag_queries()

            # ---------------- Phase K: keys -> sim + block candidates -----
            # value prefetch pool spans K+W so DMA never starves after keys
            VB = 4                      # value tiles per DMA
            CW = 4                      # wexp chunk width (value tiles)
            pV_cm = tc.tile_pool(name="poolV", bufs=2)
            pV = pV_cm.__enter__()
            vgroups = {}

            def issue_vals(g):
                vtb = pV.tile([128, VB, C], F32R, tag="vtb", bufs=4)
                nc.sync.dma_start(
                    out=vtb,
                    in_=vals[g * VB * 128:(g + 1) * VB * 128].rearrange(
                        "(v p) c -> p v c", p=128).bitcast(F32R))
                vgroups[g] = vtb

            pKx_cm = tc.tile_pool(name="poolKx", bufs=1)
            pKx = pKx_cm.__enter__()
            if True:
                # Two passes over the keys. Pass 1 streams all 16 blocks
                # once to compute per-key 1/||k|| broadcast tiles (psbS_all)
                # — this has no dependency on the queries, so it fills the
                # entire AllGather window with DMA + PE + ACT/DVE work.
                # Pass 2 re-streams the keys (DMA has spare capacity here)
                # for a short, dependency-free sim pipeline.
                psbS_all = pKx.tile([128, NMB, mb], F32, name="psbS_all",
                                   bufs=1)
                # per-block key norms land as rows of normsAll; ONE batched
                # DVE reciprocal then serves all blocks (DVE reciprocal cost
                # scales with free size only: [1,512] costs the same 3.3us
                # as [16,512], so per-block recips would burn 53us of DVE)
                rinvAll = pKx.tile([NMB, mb], F32, name="rinvAll", bufs=1)
                # eyerow[0, mbi*NMB+k] == (k==mbi): one-hot rows, all on
                # partition 0, for packing norm rows into PSUM partitions
                eyerow = pKx.tile([1, NMB * NMB], F32, name="eyerow", bufs=1)
                nc.vector.memset(eyerow, 0.0)
                for mbi in range(NMB):
                    nc.vector.memset(
                        eyerow[0:1, mbi * NMB + mbi:mbi * NMB + mbi + 1],
                        1.0)
                pK1_cm = tc.tile_pool(name="poolK1", bufs=2)
                pK1 = pK1_cm.__enter__()
                ps1_cm = tc.tile_pool(name="psumK1", bufs=1, space="PSUM")
                ps1 = ps1_cm.__enter__()
                psnPack = ps1.tile([NMB, mb], F32, name="psnPack", bufs=1)
                # one-hot selectors: sel_all[k, mbi*128+p] == (k == mbi), so
                # matmul(lhsT=sel_all[:, mbi-slice], rhs=rinvAll) replicates
                # row mbi across all 128 partitions exactly (fp32, 1.0*x)
                sel_all = pKx.tile([NMB, NMB * 128], F32, name="sel_all",
                                  bufs=1)
                for mbi in range(NMB):
                    nc.vector.tensor_scalar_mul(
                        sel_all[:, ts(mbi, 128)], ones_hw[0:NMB, 0:128],
                        identity[0:NMB, mbi:mbi + 1])

                def key_norms(mbi):
                    ktb = pK1.tile([128, CT, mb], F32, tag="ktb1", bufs=2)
                    nc.sync.dma_start(
                        out=ktb,
                        in_=keysT[:, mbi * mb:(mbi + 1) * mb].rearrange(
                            "(ct p) m -> p ct m", p=128))
                    # squares -> ksq (f32r-rounded, so the f32r norm matmul
                    # sums exactly-representable terms: ~8e-6 norm error)
                    ksq = pK1.tile([128, CT, mb], F32R, tag="ksq", bufs=2)
                    for ct in range(CT):
                        if ct == 0:
                            nc.vector.tensor_mul(ksq[:, ct], ktb[:, ct],
                                                 ktb[:, ct])
                        else:
                            nc.scalar.square(ksq[:, ct], ktb[:, ct])
                    # norms^2: contract partitions via ones f32r matmul
                    psn = ps1.tile([1, mb], F32, tag="psn", bufs=2)
                    for ct in range(CT):
                        nc.tensor.matmul(psn, lhsT=ones_col_r,
                                         rhs=ksq[:, ct],
                                         start=(ct == 0), stop=(ct == CT - 1),
                                         skip_group_check=True)
                    nrow = pK1.tile([1, mb], F32, tag="nrow", bufs=2)
                    nc.scalar.sqrt(nrow, psn)
                    # stack row mbi of the norm matrix via an exact fp32
                    # outer product with a one-hot column
                    nc.tensor.matmul(
                        psnPack, lhsT=eyerow[0:1, ts(mbi, NMB)], rhs=nrow,
                        start=(mbi == 0), stop=(mbi == NMB - 1),
                        skip_group_check=True)

                def bcast_rinv(mbi):
                    # broadcast rinv row to 128 partitions on the PE in FULL
                    # fp32 (1.0 * x is exact; f32r here would round the
                    # per-key scale and flip rank-32/33 boundaries)
                    psb = ps1.tile([128, mb], F32, tag="psb", bufs=2)
                    nc.tensor.matmul(psb,
                                     lhsT=sel_all[:, ts(mbi, 128)],
                                     rhs=rinvAll,
                                     start=True, stop=True,
                                     skip_group_check=True)
                    nc.scalar.copy(psbS_all[:, mbi], psb)

                def sim_block(mbi, ktb):
                    for bt in range(BT):
                        psim = ps2.tile([BTW, mb], F32, tag="psim", bufs=4)
                        for dt in range(CT):
                            nc.tensor.matmul(
                                psim, lhsT=qTr[:, dt, ts(bt, BTW)],
                                rhs=ktb[:, dt],
                                start=(dt == 0), stop=(dt == CT - 1),
                                skip_group_check=True)
                        # evict + normalize by 1/||k|| in one DVE op
                        sblk = sim[bt][:, ts(mbi, mb)]
                        nc.vector.tensor_mul(sblk, psim, psbS_all[:, mbi])
                        nc.vector.max(
                            cand[bt][:, mbi * KPB:mbi * KPB + KPB], sblk)

                for mbi in range(NMB):
                    key_norms(mbi)
                nc.vector.reciprocal(rinvAll, psnPack)
                for mbi in range(NMB):
                    bcast_rinv(mbi)
                ps1_cm.__exit__(None, None, None)
                pK1_cm.__exit__(None, None, None)
                pK2_cm = tc.tile_pool(name="poolK2", bufs=2)
                pK2 = pK2_cm.__enter__()
                ps2_cm = tc.tile_pool(name="psumK2", bufs=1, space="PSUM")
                ps2 = ps2_cm.__enter__()
                unpack_queries()
                ktb2s = {}

                def key_load2(mbi):
                    ktb2 = pK2.tile([128, CT, mb], F32R, tag="ktb2", bufs=4)
                    nc.sync.dma_start(
                        out=ktb2,
                        in_=keysT[:, mbi * mb:(mbi + 1) * mb].rearrange(
                            "(ct p) m -> p ct m", p=128).bitcast(F32R))
                    ktb2s[mbi] = ktb2

                PF = 2
                for mbi in range(PF):
                    key_load2(mbi)
                for mbi in range(NMB):
                    if mbi + PF < NMB:
                        key_load2(mbi + PF)
                    if mbi >= NMB - 4:
                        issue_vals(mbi - (NMB - 4))
                    sim_block(mbi, ktb2s.pop(mbi))
                ps2_cm.__exit__(None, None, None)
                pK2_cm.__exit__(None, None, None)
            pKx_cm.__exit__(None, None, None)

            # ---------------- Phase Q: query norms ----------------
            with (
                tc.tile_pool(name="poolQ", bufs=2) as pQ,
                tc.tile_pool(name="psumQ", bufs=1, space="PSUM") as psQ,
            ):
                psum_ssq = psQ.tile([1, B], F32, tag="ssq")
                for ct in range(CT):
                    qsq = pQ.tile([128, B], F32R, tag="qsq")
                    nc.scalar.square(qsq, qT[ct])
                    nc.tensor.matmul(psum_ssq, lhsT=ones_col_r, rhs=qsq,
                                     start=(ct == 0), stop=(ct == CT - 1))
                qn_row = pQ.tile([1, B], F32, tag="qn_row", bufs=1)
                nc.scalar.sqrt(qn_row, psum_ssq)
                ri_row = pQ.tile([1, B], F32, tag="ri_row", bufs=1)
                nc.vector.reciprocal(ri_row, qn_row)
                for bt in range(BT):
                    psum_rt = psQ.tile([BTW, 1], F32, tag="rt")
                    nc.tensor.matmul(
                        psum_rt, lhsT=ri_row[0:1, ts(bt, BTW)],
                        rhs=ones_col[0:1, 0:1], start=True, stop=True)
                    nc.vector.tensor_copy(rinv[bt], psum_rt)

            # ---------------- Phase G + W: top-K stats, dense matmul2 -----
            cd_in = dram.tile([B, K], F32)
            cd_out = dram.tile([n_cores, B, K], F32, addr_space=CC_AS)
            with (
                tc.tile_pool(name="poolW", bufs=2) as pW,
                tc.tile_pool(name="psumW", bufs=1, space="PSUM") as psW,
                tc.tile_pool(name="poolG", bufs=1) as pG,
            ):
                # G1: local top-K per bt -> AllGather (bt chains interleaved
                # so the DVE pipeline stays full)
                locs = [pG.tile([BTW, R * 8], F32, tag=f"loc{bt}",
                                name=f"loc{bt}") for bt in range(BT)]
                scr2s = [pG.tile([BTW, NMB * KPB], F32, tag=f"scr2{bt}",
                                 name=f"scr2{bt}") for bt in range(BT)]
                curs = [cand[bt] for bt in range(BT)]
                for r in range(R):
                    for bt in range(BT):
                        nc.vector.max(locs[bt][:, r * 8:(r + 1) * 8],
                                      curs[bt])
                    if r < R - 1:
                        for bt in range(BT):
                            nc.vector.match_replace(
                                scr2s[bt],
                                in_to_replace=locs[bt][:, r * 8:(r + 1) * 8],
                                in_values=curs[bt], imm_value=NEG)
                            curs[bt] = scr2s[bt]
                for bt in range(BT):
                    nc.sync.dma_start(out=cd_in[ts(bt, BTW), :],
                                      in_=locs[bt][:, 0:K])
                nc.gpsimd.collective_compute(
                    "AllGather", ALU.bypass, replica_groups=RG,
                    ins=[cd_in.opt()], outs=[cd_out.opt()])

                issue_vals(4)
                issue_vals(5)

                # G2: global top-K + softmax stats. bt chains interleaved;
                # Exp/Ln activations grouped to avoid ACT table thrash.
                gcs, scr3s = [], []
                for bt in range(BT):
                    gc = pG.tile([BTW, n_cores * K], F32, tag=f"gc{bt}",
                                 name=f"gc{bt}")
                    nc.scalar.dma_start(
                        out=gc,
                        in_=cd_out[:, ts(bt, BTW), :].rearrange(
                            "r b k -> b r k"))
                    gcs.append(gc)
                    scr3s.append(pG.tile([BTW, n_cores * K], F32,
                                         tag=f"scr3{bt}", name=f"scr3{bt}"))
                curs = gcs
                for r in range(R):
                    for bt in range(BT):
                        nc.vector.max(g32[bt][:, r * 8:(r + 1) * 8],
                                      curs[bt])
                    if r < R - 1:
                        for bt in range(BT):
                            nc.vector.match_replace(
                                scr3s[bt],
                                in_to_replace=g32[bt][:, r * 8:(r + 1) * 8],
                                in_values=curs[bt], imm_value=NEG)
                        curs = scr3s
                # stats: nb = -gmax*rinv ; Z = sum exp((g - gmax)*rinv)
                nbs, zzs = [], []
                for bt in range(BT):
                    nb = pG.tile([BTW, 1], F32, tag=f"nb{bt}",
                                 name=f"nb{bt}")
                    nc.vector.tensor_mul(nb, g32[bt][:, 0:1], rinv[bt])
                    nc.vector.tensor_scalar_mul(nb, nb, -1.0)
                    nbs.append(nb)
                for bt in range(BT):
                    ex = pG.tile([BTW, K], F32, tag="ex")
                    zz = pG.tile([BTW, 1], F32, tag=f"zz{bt}",
                                 name=f"zz{bt}")
                    nc.scalar.activation(ex, g32[bt][:, 0:K], AF.Exp,
                                         bias=nbs[bt], scale=rinv[bt],
                                         accum_out=zz)
                    zzs.append(zz)
                lnzs = []
                for bt in range(BT):
                    lnz = pG.tile([BTW, 1], F32, tag=f"lnz{bt}",
                                  name=f"lnz{bt}")
                    nc.scalar.activation(lnz, zzs[bt], AF.Ln)
                    lnzs.append(lnz)
                for bt in range(BT):
                    nc.vector.tensor_sub(bias2[bt], nbs[bt], lnzs[bt])

                # W: dense weights + matmul2
                pmB = [psW.tile([BTW, C], F32, tag=f"pmB{bt}",
                                name=f"pmB{bt}") for bt in range(BT)]
                for mt in range(MT):
                    g = mt // VB
                    if mt % VB == 0 and g + 3 not in vgroups and \
                            (g + 3) * VB * 128 < MS:
                        issue_vals(g + 3)
                    if mt % CW == 0:
                        # weights quantized to bf16 after the f32 threshold
                        # compare: 2^-9 relative on softmax weights is far
                        # below the tolerance, and bf16 transposes run the
                        # PE at 1 cyc/row with fast (FWL) weight loads.
                        weF = [pW.tile([BTW, CW * 128], F32R, tag=f"weF{bt}",
                                       bufs=2, name=f"weF{bt}")
                               for bt in range(BT)]
                        we = [pW.tile([BTW, CW * 128], mybir.dt.bfloat16,
                                      tag=f"we{bt}", bufs=2,
                                      name=f"we{bt}")
                              for bt in range(BT)]
                        for bt in range(BT):
                            schunk = sim[bt][:, mt * 128:(mt + CW) * 128]
                            nc.scalar.activation(weF[bt], schunk, AF.Exp,
                                                 bias=bias2[bt],
                                                 scale=rinv[bt])
                            nc.vector.scalar_tensor_tensor(
                                out=we[bt], in0=schunk,
                                scalar=g32[bt][:, K - 1:K], in1=weF[bt],
                                op0=ALU.is_ge, op1=ALU.mult)
                    vt = vgroups[g][:, mt % VB]
                    off = (mt % CW) * 128
                    pwt = psW.tile([128, B], mybir.dt.bfloat16, tag="pwt",
                                   bufs=4)
                    for bt in range(BT):
                        nc.tensor.matmul(
                            pwt[:, ts(bt, BTW)],
                            lhsT=we[bt][:, off:off + 128],
                            rhs=identity_h[0:BTW, 0:BTW], is_transpose=True,
                            start=True, stop=True, skip_group_check=True)
                    wT = pW.tile([128, B], F32R, tag="wT", bufs=3)
                    if mt % 2 == 0:
                        nc.vector.tensor_copy(wT, pwt)
                    else:
                        nc.scalar.copy(wT, pwt)
                    for bt in range(BT):
                        nc.tensor.matmul(
                            pmB[bt], lhsT=wT[:, ts(bt, BTW)], rhs=vt,
                            start=(mt == 0), stop=(mt == MT - 1),
                            skip_group_check=True)
                for bt in range(BT):
                    nc.any.tensor_copy(mrow[bt], pmB[bt])
            pV_cm.__exit__(None, None, None)

            # ---------------- Phase O: reduce-scatter + broadcast out -----
            # two C-halves: RS of half B overlaps the broadcast + write of
            # half A, shortening the serial tail
            CH = C // 2
            CTH2 = CT // 2
            mb_dram = [dram.tile([B, CH], F32, name=f"mb_dram{h}")
                       for h in range(2)]
            rs_out = [dram.tile([BS, CH], F32, name=f"rs_out{h}")
                      for h in range(2)]
            with (
                tc.tile_pool(name="poolO", bufs=2) as pO,
                tc.tile_pool(name="psumO", bufs=1, space="PSUM") as psO,
            ):
                for h in range(2):
                    for bt in range(BT):
                        nc.sync.dma_start(
                            out=mb_dram[h][ts(bt, BTW), :],
                            in_=mrow[bt][:, h * CH:(h + 1) * CH])
                    nc.gpsimd.collective_compute(
                        "ReduceScatter", ALU.add, replica_groups=RG,
                        ins=[mb_dram[h].opt()], outs=[rs_out[h].opt()])
                mmy = [pO.tile([BS, CH], F32, tag=f"mmy{h}",
                               name=f"mmy{h}", bufs=1) for h in range(2)]
                for h in range(2):
                    nc.scalar.dma_start(out=mmy[h], in_=rs_out[h])
                    for dt2 in range(CTH2):
                        dt = h * CTH2 + dt2
                        pmt = psO.tile([128, BS], F32, tag="pmt", bufs=2)
                        nc.tensor.matmul(
                            pmt, lhsT=mmy[h][:, ts(dt2, 128)],
                            rhs=identity[0:BS, 0:BS], is_transpose=True,
                            start=True, stop=True, skip_group_check=True)
                        nc.any.tensor_copy(mTmy[dt], pmt)
                    for b2 in range(BS // 2):
                        ot = pO.tile([128, 2, CTH2, HW], F32, tag="ot",
                                     bufs=3)
                        for bb in range(2):
                            b = 2 * b2 + bb
                            for dt2 in range(CTH2):
                                dt = h * CTH2 + dt2
                                col = mTmy[dt][:, b:b + 1]
                                if (dt + bb) % 2 == 0:
                                    nc.vector.tensor_scalar_mul(
                                        ot[:, bb, dt2], ones_hw, col)
                                else:
                                    nc.scalar.mul(ot[:, bb, dt2], ones_hw,
                                                  col)
                        for bb in range(2):
                            nc.sync.dma_start(
                                out=out[2 * b2 + bb,
                                        h * CH:(h + 1) * CH].rearrange(
                                    "(ct p) hw -> p ct hw", p=128),
                                in_=ot[:, bb])

    nc.compile()
    return nc


_CACHE = {}
TRACE = False
TRACE_DIR = None
LAST_RESULT = None


def _get(shape_key):
    if shape_key not in _CACHE:
        _CACHE[shape_key] = build(*shape_key)
    return _CACHE[shape_key]


def kernel(x, keys, values, topk, **_ignored):
    K = int(np.asarray(topk))
    B, C, H, W = x.shape
    M, D = keys.shape
    HW = H * W
    nc = _get((B, C, HW, M, K, N_CORES))
    BS, MS = B // N_CORES, M // N_CORES
    x3 = np.ascontiguousarray(x.reshape(B, C, HW)).astype(np.float32,
                                                          copy=False)
    keysT = np.ascontiguousarray(keys.T).astype(np.float32, copy=False)
    values = np.ascontiguousarray(values).astype(np.float32, copy=False)
    in_maps = [{
        "xs": x3[c * BS:(c + 1) * BS],
        "keysT": np.ascontiguousarray(keysT[:, c * MS:(c + 1) * MS]),
        "vals": values[c * MS:(c + 1) * MS],
    } for c in range(N_CORES)]
    global LAST_RESULT
    res = run_bass_kernel_spmd(nc, in_maps, core_ids=list(range(N_CORES)),
                               trace=TRACE, tmpdir=TRACE_DIR)
    LAST_RESULT = res
    outs = [res.results[c]["out"] for c in range(N_CORES)]
    return np.concatenate(outs, axis=0).reshape(B, C, H, W)


# revision 60
# speedup vs baseline: 1.1928x; 1.0478x over previous
"""Trainium2 Bass kernel: KV-memory retrieval (pool -> cosine kNN -> softmax gather).

Strategy (8 cores): shard the 65536-slot memory across cores (8192 keys/values
each) and the 256-image batch across cores (32 each) for pooling + output.
Keys are pre-transposed host-side to [C, M] so each core DMAs c-partitioned
kT tiles directly (no PE transposes for keys).

Per core, single SPMD launch:
  1. pool its x shard -> qT columns; two chunked AllGathers (first 16
     batches AG'd while the second half of x still streams in).
     Batch-tile permutation: tile A col r*16+j == global batch r*32+j,
     tile B col r*16+j == global batch r*32+16+j (undone at the mrow
     scatter before ReduceScatter).
  2. stream kT blocks [128c, 512m]: squares (ACT/DVE), norm via f32r
     ones-matmul over partitions, sqrt/recip, PE row-broadcast, DVE
     prescale -> normalized kTn (f32r); f32r matmul1 qT.T @ kTn ->
     sim [256, 8192]; per-block top-8 candidates (max8)
  3. local top-32 -> AllGather candidates -> global top-32 (sorted),
     threshold t, softmax stats gmax / Z (exp with per-partition
     scale/bias)
  4. dense w = exp(sim*rinv + bias) * (sim >= t)  (1/Z folded into bias),
     stored f32r
  5. matmul2 (f32r): values.T @ w -> partial matched.T; values are
     bitcast f32->f32r (no copy)
  6. transpose -> [256, 512], ReduceScatter(add) -> own batch shard
  7. broadcast over 784 spatial positions, DMA out [32, 512, 784]

f32r (single-pass fp32 on the PE) is safe here: measured sim noise ~1e-6
in cos units vs ~4e-4 gaps between rank 32/33. Selection is done on raw
r = q_sum . k_norm (scale-invariant per batch row); 1/||q|| enters only
through the exp scale. Mean /784 cancels everywhere.
"""

import math

import numpy as np

import concourse.bacc as bacc
import concourse.mybir as mybir
import concourse.tile as tile
from concourse.bass import ts
from concourse.bass_utils import run_bass_kernel_spmd
from concourse.masks import make_identity

F32 = mybir.dt.float32
F32R = mybir.dt.float32r
AF = mybir.ActivationFunctionType
ALU = mybir.AluOpType

N_CORES = 8
NEG = -3.0e38


def build(B=256, C=512, HW=784, M=65536, K=32, n_cores=N_CORES, mb=512):
    """Build + bacc-compile the SPMD program. Returns nc."""
    BS = B // n_cores          # batches per core
    HB = BS // 2               # half-batch chunk for split AllGather
    MS = M // n_cores          # memory slots per core
    CT = C // 128              # channel tiles (also contraction tiles)
    BT = B // 128 if B >= 128 else 1
    BTW = 128 if B >= 128 else B   # batch-tile width
    assert B % BTW == 0 and C % 128 == 0 and M % (n_cores * mb) == 0
    NMB = MS // mb             # key blocks per core
    R = math.ceil(K / 8)       # max8 rounds for exact top-K
    KPB = 8                    # top-8 per 512-block (validated sufficient)
    MT = MS // 128             # value tiles
    RG = [list(range(n_cores))]
    CC_AS = "Shared" if n_cores > 4 else "Local"

    nc = bacc.Bacc("TRN2", target_bir_lowering=False, debug=False,
                   num_devices=n_cores)

    xs = nc.dram_tensor("xs", [BS, C, HW], F32, kind="ExternalInput").ap()
    keysT = nc.dram_tensor("keysT", [C, MS], F32, kind="ExternalInput").ap()
    vals = nc.dram_tensor("vals", [MS, C], F32, kind="ExternalInput").ap()
    out = nc.dram_tensor("out", [BS, C, HW], F32, kind="ExternalOutput").ap()

    with tile.TileContext(nc) as tc:
        with (
            tc.tile_pool(name="consts", bufs=1) as consts,
            tc.tile_pool(name="persist", bufs=1) as persist,
            tc.tile_pool(name="dram", bufs=1, space="DRAM") as dram,
        ):
            identity = consts.tile([128, 128], F32)
            make_identity(nc, identity)
            identity_r = consts.tile([128, 128], F32R)
            nc.vector.tensor_copy(identity_r, identity)
            ones_col = consts.tile([128, 1], F32)
            nc.vector.memset(ones_col, 1.0)
            ones_col_r = consts.tile([128, 1], F32R)
            nc.vector.tensor_copy(ones_col_r, ones_col)
            ones_row = consts.tile([1, 128], F32)
            nc.vector.memset(ones_row, 1.0)
            ones_row_r = consts.tile([1, 128], F32R)
            nc.vector.tensor_copy(ones_row_r, ones_row)
            ones_hw = consts.tile([128, HW], F32)
            nc.vector.memset(ones_hw, 1.0)
            identity_h = consts.tile([128, 128], mybir.dt.bfloat16)
            nc.vector.tensor_copy(identity_h, identity)

            sim = [persist.tile([BTW, MS], F32, name=f"sim{i}")
                   for i in range(BT)]
            qTt = persist.tile([128, CT, B], F32, name="qTt")
            qT = [qTt[:, i] for i in range(CT)]
            qTr = persist.tile([128, CT, B], F32R, name="qTr")
            qTl = [persist.tile([128, BS], F32, name=f"qTl{i}")
                   for i in range(CT)]
            cand = [persist.tile([BTW, NMB * KPB], F32, name=f"cand{i}")
                    for i in range(BT)]
            rinv = [persist.tile([BTW, 1], F32, name=f"rinv{i}")
                    for i in range(BT)]
            bias2 = [persist.tile([BTW, 1], F32, name=f"bias2{i}")
                     for i in range(BT)]
            g32 = [persist.tile([BTW, R * 8], F32, name=f"g32{i}")
                   for i in range(BT)]
            mrow = [persist.tile([BTW, C], F32, name=f"mrow{i}")
                    for i in range(BT)]
            mTmy = [persist.tile([128, BS], F32, name=f"mTmy{i}")
                    for i in range(CT)]

            # AllGather staging (queries). A single AG after pooling: an
            # early chunked AG measurably steals x-read DMA bandwidth and
            # its second chunk lands late (rank skew), stalling the PE
            # queue; key_prep needs no queries, so it fills the AG window.
            qag_in = dram.tile([C, BS], F32, name="qag_in")
            qag_out = dram.tile([n_cores, C, BS], F32, addr_space=CC_AS,
                                name="qag_out")

            def ag_queries():
                for ct in range(CT):
                    nc.scalar.dma_start(out=qag_in[ts(ct, 128), :],
                                        in_=qTl[ct])
                nc.gpsimd.collective_compute(
                    "AllGather", ALU.bypass, replica_groups=RG,
                    ins=[qag_in.opt()], outs=[qag_out.opt()])

            def unpack_queries():
                # on the SP queue: SP would self-block on ktb slot pacing
                # here anyway, while ACT must keep running key_prep work
                # through the AllGather window
                for r in range(n_cores):
                    nc.sync.dma_start(
                        out=qTt[:, :, r * BS:(r + 1) * BS],
                        in_=qag_out[r].rearrange("(ct p) b -> p ct b",
                                                 p=128))
                nc.vector.tensor_copy(qTr, qTt)

            # ---------------- Phase P: pool x -> qT local ----------------
            with (
                tc.tile_pool(name="poolP", bufs=3) as pP,
            ):
                hw_a = 0
                for a in range(int(math.isqrt(HW)), 1, -1):
                    if HW % a == 0:
                        hw_a = a
                        break
                CTH = CT // 2
                for b in range(BS):
                    if b % 2 == 0:
                        xt2 = pP.tile([128, 2, CT, HW], F32, tag="xt2",
                                      bufs=3)
                        nc.sync.dma_start(
                            out=xt2,
                            in_=xs[b:b + 2].rearrange(
                                "b (ct p) hw -> p b ct hw", p=128))
                    xt = xt2[:, b % 2]
                    if hw_a > 1:
                        xp = pP.tile([128, CTH, HW // hw_a], F32, tag="xp")
                        nc.vector.tensor_reduce(
                            out=xp,
                            in_=xt[:, 0:CTH].rearrange(
                                "p ct (a b) -> p ct a b", a=HW // hw_a),
                            axis=mybir.AxisListType.X, op=ALU.add)
                        xq = pP.tile([128, CTH], F32, tag="xq")
                        nc.vector.tensor_reduce(
                            out=xq, in_=xp,
                            axis=mybir.AxisListType.X, op=ALU.add)
                    else:
                        xq = pP.tile([128, CTH], F32, tag="xq")
                        nc.vector.tensor_reduce(
                            out=xq, in_=xt[:, 0:CTH],
                            axis=mybir.AxisListType.X, op=ALU.add)
                    for ct in range(CTH):
                        nc.vector.tensor_copy(qTl[ct][:, b:b + 1],
                                              xq[:, ct:ct + 1])
                    for ct in range(CTH, CT):
                        xsc = pP.tile([128, HW], F32, tag="xsc")
                        nc.scalar.activation(
                            xsc, xt[:, ct], AF.Copy,
                            accum_out=qTl[ct][:, b:b + 1])
                ag_queries()

            # ---------------- Phase K: keys -> sim + block candidates -----
            # value prefetch pool spans K+W so DMA never starves after keys
            VB = 4                      # value tiles per DMA
            CW = 4                      # wexp chunk width (value tiles)
            pV_cm = tc.tile_pool(name="poolV", bufs=2)
            pV = pV_cm.__enter__()
            vgroups = {}

            def issue_vals(g):
                vtb = pV.tile([128, VB, C], F32R, tag="vtb", bufs=4)
                nc.sync.dma_start(
                    out=vtb,
                    in_=vals[g * VB * 128:(g + 1) * VB * 128].rearrange(
                        "(v p) c -> p v c", p=128).bitcast(F32R))
                vgroups[g] = vtb

            with (
                tc.tile_pool(name="poolK", bufs=2) as pK,
                tc.tile_pool(name="psumK", bufs=1, space="PSUM") as psK,
            ):
                # Per-key 1/||k|| with the reciprocal batched per GROUP of
                # 4 blocks: DVE reciprocal cost scales with free size only
                # ([1,512] costs the same 3.3us as [4,512]), so per-block
                # recips would burn 53us of DVE.
                GRP = 4
                NG = NMB // GRP
                # eyerow[0, j*GRP+k] == (k==j): one-hot rows on partition 0
                eyerow = pK.tile([1, GRP * GRP], F32, name="eyerow", bufs=1)
                nc.vector.memset(eyerow, 0.0)
                for j in range(GRP):
                    nc.vector.memset(
                        eyerow[0:1, j * GRP + j:j * GRP + j + 1], 1.0)
                # sel4[k, j*128+p] == (k==j): replicates row j of a [4, mb]
                # tile across 128 partitions via an exact fp32 matmul
                sel4 = pK.tile([GRP, GRP * 128], F32, name="sel4", bufs=1)
                for j in range(GRP):
                    nc.vector.tensor_scalar_mul(
                        sel4[:, ts(j, 128)], ones_hw[0:GRP, 0:128],
                        identity[0:GRP, j:j + 1])

                ktbs = {}
                psbSs = {}
                psnPacks = {}
                rinvGs = {}

                def key_prep(mbi):
                    j = mbi % GRP
                    ktb = pK.tile([128, CT, mb], F32R, tag="ktb", bufs=6)
                    nc.sync.dma_start(
                        out=ktb,
                        in_=keysT[:, mbi * mb:(mbi + 1) * mb].rearrange(
                            "(ct p) m -> p ct m", p=128).bitcast(F32R))
                    ktbs[mbi] = ktb
                    # squares -> ksq (f32r-rounded, so the f32r norm matmul
                    # sums exactly-representable terms: ~8e-6 norm error)
                    ksq = pK.tile([128, CT, mb], F32R, tag="ksq", bufs=2)
                    for ct in range(CT):
                        if ct == 0:
                            nc.vector.tensor_mul(ksq[:, ct], ktb[:, ct],
                                                 ktb[:, ct])
                        else:
                            nc.scalar.square(ksq[:, ct], ktb[:, ct])
                    # norms^2: contract partitions via ones f32r matmul
                    psn = psK.tile([1, mb], F32, tag="psn", bufs=1)
                    for ct in range(CT):
                        nc.tensor.matmul(psn, lhsT=ones_col_r,
                                         rhs=ksq[:, ct],
                                         start=(ct == 0), stop=(ct == CT - 1),
                                         skip_group_check=True)
                    nrow = pK.tile([1, mb], F32, tag="nrow", bufs=2)
                    nc.scalar.sqrt(nrow, psn)
                    # stack as row j of the group norm matrix (exact fp32
                    # outer product with a one-hot column)
                    if j == 0:
                        psnPackG = psK.tile([GRP, mb], F32, tag="psnPackG",
                                            bufs=2, name="psnPackG")
                        psnPacks[mbi // GRP] = psnPackG
                    else:
                        psnPackG = psnPacks[mbi // GRP]
                    nc.tensor.matmul(
                        psnPackG, lhsT=eyerow[0:1, ts(j, GRP)], rhs=nrow,
                        start=(j == 0), stop=(j == GRP - 1),
                        skip_group_check=True)

                def group_rinv(g):
                    rinvG = pK.tile([GRP, mb], F32, tag="rinvG", bufs=2)
                    nc.vector.reciprocal(rinvG, psnPacks.pop(g))
                    rinvGs[g] = rinvG

                def bcast_rinv(mbi):
                    # broadcast rinv row across 128 partitions on the PE in
                    # FULL fp32 (1.0 * x is exact; f32r here would round the
                    # per-key scale and flip rank-32/33 boundaries)
                    j = mbi % GRP
                    psb = psK.tile([128, mb], F32, tag="psb", bufs=2)
                    nc.tensor.matmul(psb, lhsT=sel4[:, ts(j, 128)],
                                     rhs=rinvGs[mbi // GRP],
                                     start=True, stop=True,
                                     skip_group_check=True)
                    psbS = pK.tile([128, mb], F32, tag="psbS", bufs=8)
                    nc.scalar.copy(psbS, psb)
                    psbSs[mbi] = psbS

                def sim_block(mbi):
                    ktb = ktbs.pop(mbi)
                    psbS = psbSs.pop(mbi)
                    for bt in range(BT):
                        psim = psK.tile([BTW, mb], F32, tag="psim", bufs=3)
                        for dt in range(CT):
                            nc.tensor.matmul(
                                psim, lhsT=qTr[:, dt, ts(bt, BTW)],
                                rhs=ktb[:, dt],
                                start=(dt == 0), stop=(dt == CT - 1),
                                skip_group_check=True)
                        # evict + normalize by 1/||k|| in one DVE op
                        sblk = sim[bt][:, ts(mbi, mb)]
                        nc.vector.tensor_mul(sblk, psim, psbS)
                        nc.vector.max(
                            cand[bt][:, mbi * KPB:mbi * KPB + KPB], sblk)

                # software pipeline: sims lag preps by one group (4 blocks);
                # the first group's prep + the AllGather window overlap, and
                # the sim stream only starts once qTr is unpacked.
                LAG = GRP
                for step in range(NMB + LAG):
                    mbi = step
                    if mbi < NMB:
                        key_prep(mbi)
                        if mbi % GRP == GRP - 1:
                            g = mbi // GRP
                            group_rinv(g)
                            for j in range(GRP):
                                bcast_rinv(g * GRP + j)
                        if mbi == GRP - 1:
                            unpack_queries()
                    if step >= LAG:
                        smbi = step - LAG
                        if smbi >= NMB - 4:
                            issue_vals(smbi - (NMB - 4))
                        sim_block(smbi)

            # ---------------- Phase Q: query norms ----------------
            with (
                tc.tile_pool(name="poolQ", bufs=2) as pQ,
                tc.tile_pool(name="psumQ", bufs=1, space="PSUM") as psQ,
            ):
                psum_ssq = psQ.tile([1, B], F32, tag="ssq")
                for ct in range(CT):
                    qsq = pQ.tile([128, B], F32R, tag="qsq")
                    nc.scalar.square(qsq, qT[ct])
                    nc.tensor.matmul(psum_ssq, lhsT=ones_col_r, rhs=qsq,
                                     start=(ct == 0), stop=(ct == CT - 1))
                qn_row = pQ.tile([1, B], F32, tag="qn_row", bufs=1)
                nc.scalar.sqrt(qn_row, psum_ssq)
                ri_row = pQ.tile([1, B], F32, tag="ri_row", bufs=1)
                nc.vector.reciprocal(ri_row, qn_row)
                for bt in range(BT):
                    psum_rt = psQ.tile([BTW, 1], F32, tag="rt")
                    nc.tensor.matmul(
                        psum_rt, lhsT=ri_row[0:1, ts(bt, BTW)],
                        rhs=ones_col[0:1, 0:1], start=True, stop=True)
                    nc.vector.tensor_copy(rinv[bt], psum_rt)

            # ---------------- Phase G + W: top-K stats, dense matmul2 -----
            cd_in = dram.tile([B, K], F32)
            cd_out = dram.tile([n_cores, B, K], F32, addr_space=CC_AS)
            with (
                tc.tile_pool(name="poolW", bufs=2) as pW,
                tc.tile_pool(name="psumW", bufs=1, space="PSUM") as psW,
                tc.tile_pool(name="poolG", bufs=1) as pG,
            ):
                # G1: local top-K per bt -> AllGather (bt chains interleaved
                # so the DVE pipeline stays full)
                locs = [pG.tile([BTW, R * 8], F32, tag=f"loc{bt}",
                                name=f"loc{bt}") for bt in range(BT)]
                scr2s = [pG.tile([BTW, NMB * KPB], F32, tag=f"scr2{bt}",
                                 name=f"scr2{bt}") for bt in range(BT)]
                curs = [cand[bt] for bt in range(BT)]
                for r in range(R):
                    for bt in range(BT):
                        nc.vector.max(locs[bt][:, r * 8:(r + 1) * 8],
                                      curs[bt])
                    if r < R - 1:
                        for bt in range(BT):
                            nc.vector.match_replace(
                                scr2s[bt],
                                in_to_replace=locs[bt][:, r * 8:(r + 1) * 8],
                                in_values=curs[bt], imm_value=NEG)
                            curs[bt] = scr2s[bt]
                for bt in range(BT):
                    nc.sync.dma_start(out=cd_in[ts(bt, BTW), :],
                                      in_=locs[bt][:, 0:K])
                nc.gpsimd.collective_compute(
                    "AllGather", ALU.bypass, replica_groups=RG,
                    ins=[cd_in.opt()], outs=[cd_out.opt()])

                issue_vals(4)
                issue_vals(5)

                # G2: global top-K + softmax stats. bt chains interleaved;
                # Exp/Ln activations grouped to avoid ACT table thrash.
                gcs, scr3s = [], []
                for bt in range(BT):
                    gc = pG.tile([BTW, n_cores * K], F32, tag=f"gc{bt}",
                                 name=f"gc{bt}")
                    nc.scalar.dma_start(
                        out=gc,
                        in_=cd_out[:, ts(bt, BTW), :].rearrange(
                            "r b k -> b r k"))
                    gcs.append(gc)
                    scr3s.append(pG.tile([BTW, n_cores * K], F32,
                                         tag=f"scr3{bt}", name=f"scr3{bt}"))
                curs = gcs
                for r in range(R):
                    for bt in range(BT):
                        nc.vector.max(g32[bt][:, r * 8:(r + 1) * 8],
                                      curs[bt])
                    if r < R - 1:
                        for bt in range(BT):
                            nc.vector.match_replace(
                                scr3s[bt],
                                in_to_replace=g32[bt][:, r * 8:(r + 1) * 8],
                                in_values=curs[bt], imm_value=NEG)
                        curs = scr3s
                # stats: nb = -gmax*rinv ; Z = sum exp((g - gmax)*rinv)
                nbs, zzs = [], []
                for bt in range(BT):
                    nb = pG.tile([BTW, 1], F32, tag=f"nb{bt}",
                                 name=f"nb{bt}")
                    nc.vector.tensor_mul(nb, g32[bt][:, 0:1], rinv[bt])
                    nc.vector.tensor_scalar_mul(nb, nb, -1.0)
                    nbs.append(nb)
                for bt in range(BT):
                    ex = pG.tile([BTW, K], F32, tag="ex")
                    zz = pG.tile([BTW, 1], F32, tag=f"zz{bt}",
                                 name=f"zz{bt}")
                    nc.scalar.activation(ex, g32[bt][:, 0:K], AF.Exp,
                                         bias=nbs[bt], scale=rinv[bt],
                                         accum_out=zz)
                    zzs.append(zz)
                lnzs = []
                for bt in range(BT):
                    lnz = pG.tile([BTW, 1], F32, tag=f"lnz{bt}",
                                  name=f"lnz{bt}")
                    nc.scalar.activation(lnz, zzs[bt], AF.Ln)
                    lnzs.append(lnz)
                for bt in range(BT):
                    nc.vector.tensor_sub(bias2[bt], nbs[bt], lnzs[bt])

                # W: dense weights + matmul2
                pmB = [psW.tile([BTW, C], F32, tag=f"pmB{bt}",
                                name=f"pmB{bt}") for bt in range(BT)]
                for mt in range(MT):
                    g = mt // VB
                    if mt % VB == 0 and g + 3 not in vgroups and \
                            (g + 3) * VB * 128 < MS:
                        issue_vals(g + 3)
                    if mt % CW == 0:
                        # weights quantized to bf16 after the f32 threshold
                        # compare: 2^-9 relative on softmax weights is far
                        # below the tolerance, and bf16 transposes run the
                        # PE at 1 cyc/row with fast (FWL) weight loads.
                        weF = [pW.tile([BTW, CW * 128], F32R, tag=f"weF{bt}",
                                       bufs=2, name=f"weF{bt}")
                               for bt in range(BT)]
                        we = [pW.tile([BTW, CW * 128], mybir.dt.bfloat16,
                                      tag=f"we{bt}", bufs=2,
                                      name=f"we{bt}")
                              for bt in range(BT)]
                        for bt in range(BT):
                            schunk = sim[bt][:, mt * 128:(mt + CW) * 128]
                            nc.scalar.activation(weF[bt], schunk, AF.Exp,
                                                 bias=bias2[bt],
                                                 scale=rinv[bt])
                            nc.vector.scalar_tensor_tensor(
                                out=we[bt], in0=schunk,
                                scalar=g32[bt][:, K - 1:K], in1=weF[bt],
                                op0=ALU.is_ge, op1=ALU.mult)
                    vt = vgroups[g][:, mt % VB]
                    off = (mt % CW) * 128
                    pwt = psW.tile([128, B], mybir.dt.bfloat16, tag="pwt",
                                   bufs=4)
                    for bt in range(BT):
                        nc.tensor.matmul(
                            pwt[:, ts(bt, BTW)],
                            lhsT=we[bt][:, off:off + 128],
                            rhs=identity_h[0:BTW, 0:BTW], is_transpose=True,
                            start=True, stop=True, skip_group_check=True)
                    wT = pW.tile([128, B], F32R, tag="wT", bufs=3)
                    if mt % 2 == 0:
                        nc.vector.tensor_copy(wT, pwt)
                    else:
                        nc.scalar.copy(wT, pwt)
                    for bt in range(BT):
                        nc.tensor.matmul(
                            pmB[bt], lhsT=wT[:, ts(bt, BTW)], rhs=vt,
                            start=(mt == 0), stop=(mt == MT - 1),
                            skip_group_check=True)
                for bt in range(BT):
                    nc.any.tensor_copy(mrow[bt], pmB[bt])
            pV_cm.__exit__(None, None, None)

            # ---------------- Phase O: reduce-scatter + broadcast out -----
            # two C-halves: RS of half B overlaps the broadcast + write of
            # half A, shortening the serial tail
            CH = C // 2
            CTH2 = CT // 2
            mb_dram = [dram.tile([B, CH], F32, name=f"mb_dram{h}")
                       for h in range(2)]
            rs_out = [dram.tile([BS, CH], F32, name=f"rs_out{h}")
                      for h in range(2)]
            with (
                tc.tile_pool(name="poolO", bufs=2) as pO,
                tc.tile_pool(name="psumO", bufs=1, space="PSUM") as psO,
            ):
                for h in range(2):
                    for bt in range(BT):
                        nc.sync.dma_start(
                            out=mb_dram[h][ts(bt, BTW), :],
                            in_=mrow[bt][:, h * CH:(h + 1) * CH])
                    nc.gpsimd.collective_compute(
                        "ReduceScatter", ALU.add, replica_groups=RG,
                        ins=[mb_dram[h].opt()], outs=[rs_out[h].opt()])
                mmy = [pO.tile([BS, CH], F32, tag=f"mmy{h}",
                               name=f"mmy{h}", bufs=1) for h in range(2)]
                for h in range(2):
                    nc.scalar.dma_start(out=mmy[h], in_=rs_out[h])
                    for dt2 in range(CTH2):
                        dt = h * CTH2 + dt2
                        pmt = psO.tile([128, BS], F32, tag="pmt", bufs=2)
                        nc.tensor.matmul(
                            pmt, lhsT=mmy[h][:, ts(dt2, 128)],
                            rhs=identity[0:BS, 0:BS], is_transpose=True,
                            start=True, stop=True, skip_group_check=True)
                        nc.any.tensor_copy(mTmy[dt], pmt)
                    for b2 in range(BS // 2):
                        ot = pO.tile([128, 2, CTH2, HW], F32, tag="ot",
                                     bufs=3)
                        for bb in range(2):
                            b = 2 * b2 + bb
                            for dt2 in range(CTH2):
                                dt = h * CTH2 + dt2
                                col = mTmy[dt][:, b:b + 1]
                                if (dt + bb) % 2 == 0:
                                    nc.vector.tensor_scalar_mul(
                                        ot[:, bb, dt2], ones_hw, col)
                                else:
                                    nc.scalar.mul(ot[:, bb, dt2], ones_hw,
                                                  col)
                        for bb in range(2):
                            nc.sync.dma_start(
                                out=out[2 * b2 + bb,
                                        h * CH:(h + 1) * CH].rearrange(
                                    "(ct p) hw -> p ct hw", p=128),
                                in_=ot[:, bb])

    nc.compile()
    return nc


_CACHE = {}
TRACE = False
TRACE_DIR = None
LAST_RESULT = None


def _get(shape_key):
    if shape_key not in _CACHE:
        _CACHE[shape_key] = build(*shape_key)
    return _CACHE[shape_key]


def kernel(x, keys, values, topk, **_ignored):
    K = int(np.asarray(topk))
    B, C, H, W = x.shape
    M, D = keys.shape
    HW = H * W
    nc = _get((B, C, HW, M, K, N_CORES))
    BS, MS = B // N_CORES, M // N_CORES
    x3 = np.ascontiguousarray(x.reshape(B, C, HW)).astype(np.float32,
                                                          copy=False)
    keysT = np.ascontiguousarray(keys.T).astype(np.float32, copy=False)
    values = np.ascontiguousarray(values).astype(np.float32, copy=False)
    in_maps = [{
        "xs": x3[c * BS:(c + 1) * BS],
        "keysT": np.ascontiguousarray(keysT[:, c * MS:(c + 1) * MS]),
        "vals": values[c * MS:(c + 1) * MS],
    } for c in range(N_CORES)]
    global LAST_RESULT
    res = run_bass_kernel_spmd(nc, in_maps, core_ids=list(range(N_CORES)),
                               trace=TRACE, tmpdir=TRACE_DIR)
    LAST_RESULT = res
    outs = [res.results[c]["out"] for c in range(N_CORES)]
    return np.concatenate(outs, axis=0).reshape(B, C, H, W)
